# revision 18
# baseline (speedup 1.0000x reference)
"""Trainium2 Bass kernel for nn_NeuralVoiceDecoder (self-contained).

kernel(**inputs) takes FULL inputs (batch 32), shards batch across 8
NeuronCores (4 rows each), runs one SPMD Bass program, gathers full output.

Algorithm / layouts (verified against the reference in fp64 at ~5e-4 rel):
  (b) "phase-major":  X2[c, R] = x[256*R + c], c in [0,256) as 2 halves of
      128 partitions, R in [0, NB).  STFT frames of hop 256 become pure
      column-shifted views of (b); overlap-add is PSUM column-shifted
      accumulation of the ISTFT matmuls.
  (c) "block-major":  chunks of 125 blocks: tile[p, jc] covers sample
      m = 256*(125*cc + p) + (jc-1), halo col jc=0 -> j=-1.
  q-packed rfft: q<513 -> RE bin q ; q in [513,1024) -> IM bin q-512
      (exactly 1024 rows: 513 RE + 511 nonzero IM).

Glottal source: phase cumsum distributes over the linear upsample, so the
within-block cumulative sums are a K=3 matmul with host-precomputed prefix
matrices; block offsets come from tiny triangular matmuls (all fp32 - the
phase is numerically chaotic downstream and must stay near-exact, matching
jax's tree-structured cumsum).  pulse*mask = sin^2(pi*w)*mask with
w = v - round(v) keeps the ACT Sin argument inside +-pi/2 where it is
accurate; floor(x) = int32_convert(x - 0.5) (convert rounds to nearest).
mask sigmoid is computed as 0.5 + 0.5*tanh(x/2): tanh and sin share one
ACT table set (silu_and_others) so no per-chunk table reloads.

Perf structure: rows are software-pipelined (stage P1 = excitation build,
vector/scalar-heavy; stage P2 = STFT*filter+ISTFT, tensor-dense) with P1
emitted two rows ahead so the PE stays continuously busy (HAM clock gate
needs sustained activity for the 2.4 GHz p-state).  Big matmuls run bf16
(1 cyc/row, same as f32r at N>=256, but half the SBUF -> triple-buffered
xp2) with fp32 PSUM accumulation; quantization adds ~4e-3 rel which is
well inside the 2e-2 gate.
"""
import os
import numpy as np

SR = 16000
N_FFT = 1024
HOP = 256
N_BINS = 65
B = 32
FRAMES = 1000
L = FRAMES * HOP
T = 1001
NB = 1000
NBP = 1004
CW = 125
NCH = 8
Q = 1024
HCOL = 257
N_CORES = 8
BC = B // N_CORES            # 4 rows per core
TCH = [(0, 512), (509, 492)]  # (t0, tn); tc i fills OA R_pad chunk [512i, 512i+512)
OUT_SPLIT = 510               # R_out boundary between OA chunk 0 and 1 (crop -2)


# ----------------------------------------------------------------------------
# host constants
# ----------------------------------------------------------------------------

def _hann(n=N_FFT):
    return 0.5 * (1.0 - np.cos(2.0 * np.pi * np.arange(n) / n))


def _triple(j):
    k = 10
    m = 256 * k + j
    pos = (m + 0.5) / 256.0 - 0.5
    i0 = int(np.floor(pos))
    w = pos - i0
    out = np.zeros(3)
    out[i0 - (k - 1)] += 1.0 - w
    out[i0 + 1 - (k - 1)] += w
    return out


def build_constants():
    C = {}
    w = _hann()

    UP = np.zeros((3, HCOL))
    for jc in range(HCOL):
        UP[:, jc] = _triple(jc - 1)
    C["UPM"] = UP

    PF = np.zeros((3, HCOL))
    acc = np.zeros(3)
    for jc in range(1, HCOL):
        acc = acc + _triple(jc - 1) / SR
        PF[:, jc] = acc
    C["PF"] = PF

    wsq_int = np.array([sum(w[256 * u + c] ** 2 for u in range(4)) for c in range(256)])
    wsq_int = np.maximum(wsq_int, 1e-11)
    WA = np.zeros((2, 3, 128))
    for h in range(2):
        for c in range(128):
            cf = 128 * h + c
            WA[h, :, c] = _triple(cf) / wsq_int[cf]
    C["WA"] = WA

    m_pad = np.arange(512, 512 + L)
    wsq_true = np.zeros(L)
    for u in range(-3, 4):
        t = m_pad // 256 + u
        s = m_pad - 256 * t
        valid = (t >= 0) & (t < T) & (s >= 0) & (s < 1024)
        wsq_true[valid] += w[s[valid]] ** 2
    wsq_true = np.maximum(wsq_true, 1e-11)
    ratio = wsq_int[np.arange(L) % 256] / wsq_true
    ratio_bR = ratio.reshape(NB, 256).T
    edge_cols = [R for R in range(NB)
                 if not np.allclose(ratio_bR[:, R], 1.0, atol=1e-13)]
    assert edge_cols == [0, NB - 1], edge_cols
    C["CHI"] = np.stack([ratio_bR[:, 0], ratio_bR[:, NB - 1]], 1)  # [256, 2]

    s = np.arange(N_FFT)
    CS = np.zeros((N_FFT, Q))
    for q in range(Q):
        if q < 513:
            CS[:, q] = w * np.cos(2 * np.pi * q * s / N_FFT)
        else:
            CS[:, q] = -w * np.sin(2 * np.pi * (q - 512) * s / N_FFT)
    C["CS"] = CS

    MI = np.zeros((Q, N_FFT))
    for q in range(Q):
        if q == 0:
            MI[q] = 1.0 / N_FFT
        elif q < 512:
            MI[q] = 2.0 / N_FFT * np.cos(2 * np.pi * q * s / N_FFT)
        elif q == 512:
            MI[q] = 1.0 / N_FFT * np.cos(np.pi * s)
        else:
            MI[q] = -2.0 / N_FFT * np.sin(2 * np.pi * (q - 512) * s / N_FFT)
    MI = MI * w[None, :]
    C["MI"] = MI

    Wt = np.zeros((FRAMES, T))
    for t in range(T):
        pos = (t + 0.5) * (FRAMES / T) - 0.5
        pos = min(max(pos, 0.0), FRAMES - 1.0)
        i0 = int(np.floor(pos)); i1 = min(i0 + 1, FRAMES - 1)
        wt = pos - i0
        Wt[i0, t] += 1.0 - wt
        Wt[i1, t] += wt
    C["WT"] = Wt

    Wq = np.zeros((N_BINS, Q))
    for q in range(Q):
        bq = q if q < 513 else q - 512
        pos = (bq + 0.5) * (N_BINS / 513.0) - 0.5
        pos = min(max(pos, 0.0), N_BINS - 1.0)
        i0 = int(np.floor(pos)); i1 = min(i0 + 1, N_BINS - 1)
        wq = pos - i0
        Wq[i0, q] += 1.0 - wq
        Wq[i1, q] += wq
    C["WQ"] = Wq

    # reflect-edge permutations, deduped.  For each (h, R_pad): list of
    # (src_half, src_R, mat_index); mats stacked in C["EDGE_P"].
    def edge_src(R_pad, cf):
        if R_pad < 2:
            return 512 - (256 * R_pad + cf)
        return L - 2 - (256 * (R_pad - 1002) + cf)
    mats = []
    mat_keys = {}
    edge_plan = {}
    for R_pad in [0, 1, 1002, 1003]:
        for h in range(2):
            blocks = {}
            for c in range(128):
                cf = 128 * h + c
                msrc = edge_src(R_pad, cf)
                assert 0 <= msrc < L
                Rs, cs = divmod(msrc, 256)
                hs, csl = divmod(cs, 128)
                if (hs, Rs) not in blocks:
                    blocks[(hs, Rs)] = np.zeros((128, 128), np.float32)
                blocks[(hs, Rs)][csl, c] = 1.0
            plan = []
            for (hs, Rs), P in blocks.items():
                key = P.tobytes()
                if key not in mat_keys:
                    mat_keys[key] = len(mats)
                    mats.append(P)
                plan.append((hs, Rs, mat_keys[key]))
            edge_plan[(h, R_pad)] = plan
    C["EDGE_P"] = np.stack(mats)          # [nP, 128, 128]
    C["edge_plan"] = edge_plan

    # Wt nonzero block list per t-chunk
    wt_blocks = {}
    for tci, (t0, tn) in enumerate(TCH):
        for kk in range(NCH):
            blk = Wt[CW * kk:CW * kk + CW, t0:t0 + tn]
            if np.any(blk):
                wt_blocks.setdefault(tci, []).append(kk)
    C["wt_blocks"] = wt_blocks
    return C


def _x3_of(x):          # [n, 1000] -> [n, 3, 1000] with edge clamping
    n = x.shape[0]
    x3 = np.zeros((n, 3, FRAMES), np.float32)
    x3[:, 0, 1:] = x[:, :-1]; x3[:, 0, 0] = x[:, 0]
    x3[:, 1] = x
    x3[:, 2, :-1] = x[:, 1:]; x3[:, 2, -1] = x[:, -1]
    return x3


# ----------------------------------------------------------------------------
# device program
# ----------------------------------------------------------------------------

def build_program(C):
    import concourse.bacc as bacc
    import concourse.tile as tile
    from concourse import mybir

    F32 = mybir.dt.float32
    F32R = mybir.dt.float32r
    BF16 = mybir.dt.bfloat16
    I32 = mybir.dt.int32
    ALU = mybir.AluOpType
    ACTF = mybir.ActivationFunctionType

    nc = bacc.Bacc("TRN2", target_bir_lowering=False, debug=False)

    # ---- dram I/O
    d_f0x3 = nc.dram_tensor("f0x3", [BC, 3, FRAMES], F32, kind="ExternalInput").ap()
    d_oqx3 = nc.dram_tensor("oqx3", [BC, 3, FRAMES], BF16, kind="ExternalInput").ap()
    d_amx3 = nc.dram_tensor("amx3", [BC, 3, FRAMES], BF16, kind="ExternalInput").ap()
    d_vcur = nc.dram_tensor("vcur", [BC, FRAMES, N_BINS], BF16, kind="ExternalInput").ap()
    d_ncur = nc.dram_tensor("ncur", [BC, FRAMES, N_BINS], BF16, kind="ExternalInput").ap()
    d_noise = nc.dram_tensor("noise", [BC, NB, 256], BF16, kind="ExternalInput").ap()
    d_CS = nc.dram_tensor("CS", [Q, Q], BF16, kind="ExternalInput").ap()
    d_MI = nc.dram_tensor("MI", [Q, Q], BF16, kind="ExternalInput").ap()
    d_WQ = nc.dram_tensor("WQ", [N_BINS, Q], BF16, kind="ExternalInput").ap()
    d_WT = nc.dram_tensor("WT", [FRAMES, T], BF16, kind="ExternalInput").ap()
    d_PF = nc.dram_tensor("PF", [3, HCOL], F32, kind="ExternalInput").ap()
    d_UPM = nc.dram_tensor("UPM", [3, HCOL + 1], BF16, kind="ExternalInput").ap()
    d_WA = nc.dram_tensor("WA", [2, 3, 128], BF16, kind="ExternalInput").ap()
    d_TRI = nc.dram_tensor("TRI", [CW, CW], F32, kind="ExternalInput").ap()
    d_ID = nc.dram_tensor("IDENT", [128, 128], F32, kind="ExternalInput").ap()
    d_IDB = nc.dram_tensor("IDENTB", [128, 128], BF16, kind="ExternalInput").ap()
    nP = C["EDGE_P"].shape[0]
    d_EP = nc.dram_tensor("EDGE_P", [nP, 128, 128], BF16, kind="ExternalInput").ap()
    d_CHI = nc.dram_tensor("CHI", [2, 128, 2], F32, kind="ExternalInput").ap()
    d_out = nc.dram_tensor("out", [BC, NB, 256], F32, kind="ExternalOutput").ap()

    from contextlib import ExitStack
    with tile.TileContext(nc) as tc, ExitStack() as es:
        cpool = es.enter_context(tc.tile_pool(name="consts", bufs=1))
        wpool = es.enter_context(tc.tile_pool(name="work", bufs=1))
        pp = es.enter_context(tc.tile_pool(name="psum", bufs=1, space="PSUM"))

        def ctile(shape, dt, name, src=None):
            t = cpool.tile(shape, dt, name=name, tag=name)
            if src is not None:
                nc.sync.dma_start(t[:], src)
            return t

        # ---- constants in SBUF
        WQt = ctile([N_BINS, Q], BF16, "WQt", d_WQ[:])
        PFt = ctile([3, HCOL], F32, "PFt", d_PF[:])
        UPt = ctile([3, HCOL + 1], BF16, "UPt", d_UPM[:])
        WAt = [ctile([3, 128], BF16, f"WA{h}", d_WA[h]) for h in range(2)]
        TRIt = ctile([CW, CW], F32, "TRIt", d_TRI[:])
        IDt = ctile([128, 128], F32, "IDt", d_ID[:])
        IDb = ctile([128, 128], BF16, "IDb", d_IDB[:])
        EPt = [ctile([128, 128], BF16, f"EP{i}", d_EP[i]) for i in range(nP)]
        CHIt = [ctile([128, 2], F32, f"CHI{h}", d_CHI[h]) for h in range(2)]

        ones125 = ctile([CW, 1], F32, "ones125"); nc.vector.memset(ones125[:], 1.0)
        ones1x = ctile([1, CW], F32, "ones1x"); nc.vector.memset(ones1x[:], 1.0)
        zl = ctile([1, 128], BF16, "zl"); nc.vector.memset(zl[:], 0.0)
        zr = ctile([1, 512], BF16, "zr"); nc.vector.memset(zr[:], 0.0)
        zcol = ctile([128, 2], BF16, "zcol"); nc.vector.memset(zcol[:], 0.0)
        zlf = ctile([1, CW], F32, "zlf"); nc.vector.memset(zlf[:], 0.0)
        zrf = ctile([1, 8], F32, "zrf"); nc.vector.memset(zrf[:], 0.0)
        cv = {}
        for val in (0.0, 50.0, np.pi):
            t = ctile([128, 1], F32, f"cv{len(cv)}")
            nc.vector.memset(t[:], float(val))
            cv[val] = t

        CSt = [ctile([128, Q], BF16, f"CS{k}", d_CS[128 * k:128 * k + 128, :])
               for k in range(8)]
        MIt = [ctile([128, Q], BF16, f"MI{k}", d_MI[128 * k:128 * k + 128, :])
               for k in range(8)]
        # resident WT blocks (bf16), one per (tci, kk) with nonzero data
        WTt = {}
        for tci, (t0, tn) in enumerate(TCH):
            for kk in C["wt_blocks"][tci]:
                WTt[(tci, kk)] = ctile(
                    [CW, tn], BF16, f"WT{tci}_{kk}",
                    d_WT[CW * kk:CW * kk + CW, t0:t0 + tn])

        # ---- per-row pipeline
        def wt_(shape, dt, name, tag, bufs=2):
            return wpool.tile(shape, dt, name=name, tag=tag, bufs=bufs)

        def pt_(shape, name, tag, bufs=None, dt=None):
            if bufs is None:
                bufs = {'px': 1, 'mm': 4, 'tp': 1, 'oa': 2}[tag]
            return pp.tile(shape, dt or mybir.dt.float32, name=name, tag=tag,
                           bufs=bufs)

        xp2 = {}    # (b, sig, h) -> tile [128, NBP] bf16
        pre = {}    # (b, tci) -> prefilled noise-branch products
        ampw = {}   # (b, h) -> tile [128, NB] bf16
        curs = {}   # (b, tci, branch, kk) -> tile [CW, N_BINS] bf16

        def glottal_stage(b):
            # x3 tiles
            f0x3 = wt_([3, FRAMES], F32, f"f0x3_{b}", "x3f", 2)
            nc.sync.dma_start(f0x3[:], d_f0x3[b])
            oqx3 = wt_([3, FRAMES], BF16, f"oqx3_{b}", "x3o", 2)
            nc.sync.dma_start(oqx3[:], d_oqx3[b])

            # block totals G [CW, NCH] (fp32)
            g_ps = pt_([CW, 8], f"g_ps_{b}", "px")
            nc.tensor.matmul(g_ps[:], zlf[:, 0:CW], zrf[:], start=True, stop=False,
                             skip_group_check=True)
            for cc in range(NCH):
                nc.tensor.matmul(g_ps[:, cc:cc + 1], f0x3[:, CW * cc:CW * cc + CW],
                                 PFt[:, HCOL - 1:HCOL], start=False,
                                 stop=(cc == NCH - 1), skip_group_check=True)
            g_sb = wt_([CW, 8], F32, f"g_sb_{b}", "gsb", 2)
            nc.vector.tensor_copy(g_sb[:], g_ps[:])

            # P_excl = TRIstrict @ G + broadcast(base)
            h_ps = pt_([1, 8], f"h_ps_{b}", "px")
            nc.tensor.matmul(h_ps[:], ones125[:], g_sb[:], start=True, stop=True)
            h_sb = wt_([1, 8], F32, f"h_sb_{b}", "hsb", 2)
            nc.vector.tensor_copy(h_sb[:], h_ps[:])
            base = wt_([1, 8], F32, f"base_{b}", "base", 2)
            nc.vector.memset(base[:, 0:1], 0.0)
            nc.vector.tensor_tensor_scan(base[:, 1:8], h_sb[:, 0:7],
                                         zrf[0:1, 0:7], 0.0, ALU.add, ALU.bypass)
            p_ps = pt_([CW, 8], f"p_ps_{b}", "px")
            nc.tensor.matmul(p_ps[:], TRIt[:], g_sb[:], start=True, stop=False)
            nc.tensor.matmul(p_ps[:], ones1x[:], base[:], start=False, stop=True,
                             skip_group_check=True)
            p5 = wt_([CW, 8], F32, f"p5_{b}", "p5", 2)
            nc.vector.tensor_scalar(p5[:], p_ps[:], -0.5, None, ALU.add)

            # quarter-row elementwise, phase-batched so all 4 Tanh run
            # back-to-back and all 4 Sin run back-to-back (one ACT table
            # load each instead of one per switch).  Engine split:
            # ACT: ph5 (Identity+bias), oqe, th (Tanh), sn (Sin)
            # DVE: p, roq, u2, mask, varg, sm, g, sc-add
            # Pool: i32/i32v casts (round-to-nearest), marg, e
            W2 = 2 * HCOL
            ph5_, oqe_, p_, marg_, th_ = {}, {}, {}, {}, {}
            for qq in range(4):
                ccs = (2 * qq, 2 * qq + 1)
                ph5 = wt_([CW, W2], F32, f"ph5_{b}_{qq}", "ph5", 2)
                oqe = wt_([CW, W2], F32, f"oqe_{b}_{qq}", "oqe", 4)
                for i, cc in enumerate(ccs):
                    s_ps = pt_([CW, HCOL], f"s_ps_{b}_{cc}", "px")
                    nc.tensor.matmul(s_ps[:], f0x3[:, CW * cc:CW * cc + CW], PFt[:],
                                     start=True, stop=True)
                    nc.scalar.activation(ph5[:, HCOL * i:HCOL * (i + 1)], s_ps[:],
                                         ACTF.Identity, bias=p5[:, cc:cc + 1],
                                         scale=1.0)
                    o_ps = pt_([CW, HCOL + 1], f"o_ps_{b}_{cc}", "px")
                    nc.tensor.matmul(o_ps[:], oqx3[:, CW * cc:CW * cc + CW], UPt[:],
                                     start=True, stop=True)
                    nc.scalar.activation(oqe[:, HCOL * i:HCOL * (i + 1)],
                                         o_ps[:, 0:HCOL],
                                         ACTF.Copy, bias=1e-8, scale=1.0)
                i32 = wt_([CW, W2], I32, f"i32_{b}_{qq}", "i32", 2)
                nc.gpsimd.tensor_copy(i32[:], ph5[:])
                p = wt_([CW, W2], F32, f"p_{b}_{qq}", "pt", 4)
                nc.vector.scalar_tensor_tensor(p[:], ph5[:], 0.5, i32[:],
                                               ALU.add, ALU.subtract)
                marg = wt_([CW, W2], F32, f"marg_{b}_{qq}", "t1", 4)
                nc.gpsimd.tensor_tensor(marg[:], oqe[:], p[:], ALU.subtract)
                ph5_[qq], oqe_[qq], p_[qq], marg_[qq] = ph5, oqe, p, marg
            for qq in range(4):
                # mask = sigmoid(100*marg) = 0.5 + 0.5*tanh(50*marg); the 4
                # Tanh run back-to-back (one table load), and the only ACT
                # ops until the Sin below are Copy/Identity (table-free)
                th = wt_([CW, W2], F32, f"th_{b}_{qq}", "t2", 4)
                nc.scalar.activation(th[:], marg_[qq][:], ACTF.Tanh,
                                     bias=cv[0.0][0:CW, :], scale=cv[50.0][0:CW, :])
                th_[qq] = th
            for qq in range(4):
                ccs = (2 * qq, 2 * qq + 1)
                mask = wt_([CW, W2], F32, f"mask_{b}_{qq}", "t4", 1)
                nc.vector.tensor_scalar(mask[:], th_[qq][:], 0.5, 0.5,
                                        ALU.mult, ALU.add)
                roq = wt_([CW, W2], F32, f"roq_{b}_{qq}", "t58", 2)
                nc.vector.reciprocal_approx_fast(roq[:], oqe_[qq][:])
                u2 = wt_([CW, W2], F32, f"u2_{b}_{qq}", "t3", 2)
                nc.vector.scalar_tensor_tensor(u2[:], p_[qq][:], 0.5, roq[:],
                                               ALU.mult, ALU.mult)
                i32v = wt_([CW, W2], I32, f"i32v_{b}_{qq}", "i32", 2)
                nc.gpsimd.tensor_copy(i32v[:], u2[:])
                varg = wt_([CW, W2], F32, f"varg_{b}_{qq}", "t6", 2)
                nc.vector.tensor_tensor(varg[:], u2[:], i32v[:], ALU.subtract)
                sn = wt_([CW, W2], F32, f"sn_{b}_{qq}", "t7", 2)
                nc.scalar.activation(sn[:], varg[:], ACTF.Sin,
                                     bias=cv[0.0][0:CW, :], scale=cv[np.pi][0:CW, :])
                sm = wt_([CW, W2], F32, f"sm_{b}_{qq}", "t1", 4)
                nc.vector.tensor_tensor(sm[:], sn[:], mask[:], ALU.mult)
                g = wt_([CW, W2], F32, f"g_{b}_{qq}", "t58", 2)
                nc.vector.tensor_tensor(g[:], sm[:], sn[:], ALU.mult)
                e = wt_([CW, 512], BF16, f"e_{b}_{qq}", "et", 2)
                for i, cc in enumerate(ccs):
                    nc.gpsimd.tensor_tensor(e[:, 256 * i:256 * (i + 1)],
                                            g[:, HCOL * i + 1:HCOL * i + HCOL],
                                            g[:, HCOL * i:HCOL * i + 256],
                                            ALU.subtract)
                if qq == 0:
                    nc.gpsimd.memset(e[0:1, 0:1], 0.0)
                # transpose (c)->(b): e[:, 256i + 128h2 : +128] -> xp2 cols
                for i, cc in enumerate(ccs):
                    for h2 in range(2):
                        tp = pt_([128, CW], f"tp_{b}_{cc}_{h2}", "tp",
                                 dt=BF16)
                        nc.tensor.transpose(tp[:], e[:, 256 * i + 128 * h2:
                                                     256 * i + 128 * (h2 + 1)],
                                            IDb[0:CW, 0:CW])
                        dst = xp2[(b, "g", h2)]
                        if (cc + h2) % 2 == 0:
                            nc.vector.tensor_copy(dst[:, 2 + CW * cc:2 + CW * cc + CW], tp[:])
                        else:
                            nc.scalar.copy(dst[:, 2 + CW * cc:2 + CW * cc + CW], tp[:])

        def noise_stage(b):
            for cc in range(NCH):
                nb_t = wt_([CW, 256], BF16, f"nb_{b}_{cc}", "nb", 1)
                nc.sync.dma_start(nb_t[:], d_noise[b, CW * cc:CW * cc + CW, :])
                for h2 in range(2):
                    tp = pt_([128, CW], f"ntp_{b}_{cc}_{h2}", "tp", dt=BF16)
                    nc.tensor.transpose(tp[:], nb_t[:, 128 * h2:128 * (h2 + 1)],
                                        IDb[0:CW, 0:CW])
                    dst = xp2[(b, "n", h2)]
                    if (cc + h2) % 2 == 0:
                        nc.vector.tensor_copy(dst[:, 2 + CW * cc:2 + CW * cc + CW], tp[:])
                    else:
                        nc.scalar.copy(dst[:, 2 + CW * cc:2 + CW * cc + CW], tp[:])

        def edge_stage(b, sig):
            for (h, R_pad), plan in C["edge_plan"].items():
                off = 0 if R_pad < 2 else 1
                ep = pt_([128, 2], f"ep_{b}_{sig}_{h}_{R_pad}", "px")
                for i, (hs, Rs, mi) in enumerate(plan):
                    sv = xp2[(b, sig, hs)][:, 2 + Rs - off:4 + Rs - off]
                    nc.tensor.matmul(ep[:], EPt[mi][:], sv, start=(i == 0),
                                     stop=(i == len(plan) - 1),
                                     skip_group_check=True)
                dst = xp2[(b, sig, h)]
                nc.vector.tensor_copy(dst[:, R_pad:R_pad + 1], ep[:, off:off + 1])

        def ampw_stage(b):
            amx3 = wt_([3, FRAMES], BF16, f"amx3_{b}", "x3a", 2)
            nc.sync.dma_start(amx3[:], d_amx3[b])
            for h in range(2):
                aw = wt_([128, NB], BF16, f"ampw_{b}_{h}", f"ampw{h}", 3)
                for half in range(2):
                    a_ps = pt_([128, 500], f"a_ps_{b}_{h}_{half}", "px")
                    nc.tensor.matmul(a_ps[:], WAt[h][:],
                                     amx3[:, 500 * half:500 * (half + 1)],
                                     start=True, stop=True)
                    # fold chi edge correction into cols 0 and 999
                    if half == 0:
                        nc.vector.tensor_scalar(aw[:, 0:1], a_ps[:, 0:1],
                                                CHIt[h][:, 0:1], None, ALU.mult)
                        nc.vector.tensor_copy(aw[:, 1:500], a_ps[:, 1:500])
                    else:
                        nc.vector.tensor_copy(aw[:, 500:999], a_ps[:, 0:499])
                        nc.vector.tensor_scalar(aw[:, 999:1000], a_ps[:, 499:500],
                                                CHIt[h][:, 1:2], None, ALU.mult)
                ampw[(b, h)] = aw

        def cur_prefetch(b):
            for tci in range(2):
                for branch, d_cur in (("v", d_vcur), ("n", d_ncur)):
                    for kk in C["wt_blocks"][tci]:
                        cur = wt_([CW, N_BINS], BF16,
                                  f"cur_{b}_{tci}_{branch}_{kk}",
                                  f"cur{tci}{branch}{kk}", 3)
                        nc.sync.dma_start(cur[:], d_cur[b, CW * kk:CW * kk + CW, :])
                        curs[(b, tci, branch, kk)] = cur

        def P1(b):
            for sig in ("g", "n"):
                for h in range(2):
                    xp2[(b, sig, h)] = wt_([128, NBP], BF16, f"xp2{sig}{h}_{b}",
                                           f"xp2{sig}{h}", 3)
            cur_prefetch(b)
            noise_stage(b)
            edge_stage(b, "n")
            if b == 0:
                pre[(0, 0)] = n_prefill(0, 0, range(6))
            glottal_stage(b)
            edge_stage(b, "g")
            ampw_stage(b)

        def gt_stage(b, tci, branch):
            t0, tn = TCH[tci]
            gt_ps = pt_([N_BINS, tn], f"gt_ps_{b}_{tci}_{branch}", "mm")
            kks = C["wt_blocks"][tci]
            for i, kk in enumerate(kks):
                nc.tensor.matmul(gt_ps[:], curs[(b, tci, branch, kk)][:],
                                 WTt[(tci, kk)][:],
                                 start=(i == 0), stop=(i == len(kks) - 1))
            gt_sb = wt_([N_BINS, tn], BF16, f"gt_sb_{b}_{tci}_{branch}",
                        f"gtsb{branch}", 2)
            nc.scalar.copy(gt_sb[:], gt_ps[:])
            return gt_sb

        def n_branch_mm(b, tci, mm, gtn):
            # one q-tile of the noise branch: filt matmul + evict + spectrum
            # + spec*filt product (tmb, bf16 SBUF)
            t0, tn = TCH[tci]
            fp = pt_([128, tn], f"f_ps_{b}_{tci}_n_{mm}", "mm")
            nc.tensor.matmul(fp[:], WQt[:, 128 * mm:128 * (mm + 1)],
                             gtn[:], start=True, stop=True)
            fs = wt_([128, tn], BF16, f"f_sb_{b}_{tci}_n_{mm}", "fsbn", 2)
            nc.scalar.copy(fs[:], fp[:])
            sp = pt_([128, tn], f"sp_ps_{b}_{tci}_n_{mm}", "mm")
            for kk in range(8):
                rhs = xp2[(b, "n", kk % 2)][:, kk // 2 + t0:kk // 2 + t0 + tn]
                nc.tensor.matmul(sp[:], CSt[kk][:, 128 * mm:128 * (mm + 1)],
                                 rhs, start=(kk == 0), stop=(kk == 7))
            tmb = wt_([128, tn], BF16, f"tmb_{b}_{tci}_{mm}", "tmb", 6)
            nc.vector.tensor_tensor(tmb[:], sp[:], fs[:], ALU.mult)
            return tmb

        def n_prefill(b, tci, mms):
            # the noise-branch STFT*filt needs no glottal data: emitted
            # right after noise_stage(0) it fills the PE idle window while
            # row 0's glottal elementwise chain runs
            gtn = gt_stage(b, tci, "n")
            return {"gt": gtn, "tmbs": {mm: n_branch_mm(b, tci, mm, gtn)
                                        for mm in mms}}

        def filtspec_stage(b, tci, pre=None):
            t0, tn = TCH[tci]
            gt = {"g": gt_stage(b, tci, "v"),
                  "n": pre["gt"] if pre else gt_stage(b, tci, "n")}
            spec_c = []
            for mm in range(8):
                fp = pt_([128, tn], f"f_ps_{b}_{tci}_g_{mm}", "mm")
                nc.tensor.matmul(fp[:], WQt[:, 128 * mm:128 * (mm + 1)],
                                 gt["g"][:], start=True, stop=True)
                # evict filt to SBUF (only one PSUM operand allowed per
                # elementwise op; the spectrum stays in PSUM)
                fs = wt_([128, tn], BF16, f"f_sb_{b}_{tci}_g_{mm}", "fsbg", 2)
                nc.scalar.copy(fs[:], fp[:])
                sp_g = pt_([128, tn], f"sp_ps_{b}_{tci}_g_{mm}", "mm")
                for kk in range(8):
                    rhs = xp2[(b, "g", kk % 2)][:, kk // 2 + t0:kk // 2 + t0 + tn]
                    nc.tensor.matmul(sp_g[:],
                                     CSt[kk][:, 128 * mm:128 * (mm + 1)],
                                     rhs, start=(kk == 0), stop=(kk == 7))
                # tm = spec*filt on DVE (one PSUM operand each); the g+n
                # add runs on GpSimd (SBUF-only engine)
                tma = wt_([128, tn], BF16, f"tma_{b}_{tci}_{mm}", "tma", 2)
                nc.vector.tensor_tensor(tma[:], sp_g[:], fs[:], ALU.mult)
                if pre and mm in pre["tmbs"]:
                    tmb = pre["tmbs"][mm]
                else:
                    tmb = n_branch_mm(b, tci, mm, gt["n"])
                sc = wt_([128, tn + 2], BF16, f"sc_{b}_{tci}_{mm}", f"sc{mm}", 1)
                nc.vector.tensor_copy(sc[:, 0:1], zcol[:, 0:1])
                nc.vector.tensor_copy(sc[:, tn + 1:tn + 2], zcol[:, 1:2])
                nc.gpsimd.tensor_tensor(sc[:, 1:tn + 1], tma[:], tmb[:], ALU.add)
                spec_c.append(sc)
            return spec_c

        def istft_stage(b, tci, spec_c, oa):
            t0, tn = TCH[tci]
            rc = tci
            Clo = 512 * rc
            Chi = min(Clo + 512, NBP)
            for h in range(2):
                nc.tensor.matmul(oa[h][:, 0:Chi - Clo], zl[:], zr[:, 0:Chi - Clo],
                                 start=True, stop=False, skip_group_check=True)
            spans = {}
            for mm in range(8):
                u, h = divmod(mm, 2)
                lo = max(t0 + u, Clo)
                hi = min(t0 + tn + u, Chi)
                if lo >= hi:
                    continue
                if (hi - lo) % 2 == 1:
                    if rc == 0:
                        lo -= 1      # extra col reads the leading zero column
                    else:
                        hi += 1      # extra col reads the trailing zero column
                a = 1 + lo - u - t0
                bcol = 1 + hi - u - t0
                assert 0 <= a and bcol <= tn + 2 and Clo <= lo and hi <= 512 * rc + 512
                spans[mm] = (h, lo, hi, a, bcol)
            last_mm = {h: max(m for m in spans if spans[m][0] == h) for h in range(2)}
            # kk-outer: start accumulating as soon as spec_c[kk] is ready
            for kk in range(8):
                for mm, (h, lo, hi, a, bcol) in spans.items():
                    nc.tensor.matmul(oa[h][:, lo - Clo:hi - Clo],
                                     MIt[kk][:, 128 * mm:128 * (mm + 1)],
                                     spec_c[kk][:, a:bcol],
                                     start=False,
                                     stop=(kk == 7 and mm == last_mm[h]),
                                     skip_group_check=True)

        def final_stage(b, tci, oa):
            rc = tci
            # out_b = OA * ampw  (slices per rc)
            for h in range(2):
                if rc == 0:
                    w_ = OUT_SPLIT
                    ob = wt_([128, w_], BF16, f"ob_{b}_{rc}_{h}", f"ob{h}", 2)
                    nc.vector.tensor_tensor(ob[:], oa[h][:, 2:2 + w_],
                                            ampw[(b, h)][:, 0:w_], ALU.mult)
                else:
                    w_ = NB - OUT_SPLIT
                    ob = wt_([128, w_], BF16, f"ob_{b}_{rc}_{h}", f"ob{h}", 2)
                    nc.vector.tensor_tensor(ob[:], oa[h][:, 0:w_],
                                            ampw[(b, h)][:, OUT_SPLIT:NB], ALU.mult)
                # transpose to (c) pieces and DMA out
                if rc == 0:
                    bnds = [0, 128, 256, 384, OUT_SPLIT]
                    base = 0
                else:
                    bnds = [0, 128, 256, 384, NB - OUT_SPLIT]
                    base = OUT_SPLIT
                for i in range(len(bnds) - 1):
                    a, bb = bnds[i], bnds[i + 1]
                    wb = bb - a
                    tp = pt_([wb, 128], f"otp_{b}_{rc}_{h}_{i}", "tp", dt=BF16)
                    nc.tensor.transpose(tp[:], ob[:, a:bb], IDb[:])
                    st = wt_([wb, 128], F32, f"ost_{b}_{rc}_{h}_{i}", "ost", 2)
                    if i % 2 == 0:
                        nc.vector.tensor_copy(st[:], tp[:])
                    else:
                        nc.scalar.copy(st[:], tp[:])
                    nc.sync.dma_start(
                        d_out[b, base + a:base + bb, 128 * h:128 * (h + 1)], st[:])

        def P2(b):
            # final(tci0) is deferred past filtspec(tci1) so the PE rolls
            # straight from ISTFT(0) into the next chunk's matmuls while the
            # DVE does the amp-multiply eviction of OA(0)
            spec0 = filtspec_stage(b, 0, pre.pop((b, 0), None))
            oa0 = {h: pt_([128, 512], f"oa_{b}_0_{h}", "oa", 2)
                   for h in range(2)}
            istft_stage(b, 0, spec0, oa0)
            spec1 = filtspec_stage(b, 1)
            final_stage(b, 0, oa0)
            oa1 = {h: pt_([128, 512], f"oa_{b}_1_{h}", "oa", 2)
                   for h in range(2)}
            istft_stage(b, 1, spec1, oa1)
            final_stage(b, 1, oa1)

        # ---- main loop: software-pipelined rows (P1 two rows ahead of
        # the tensor-dense P2)
        P1(0)
        if BC > 1:
            P1(1)
        for b in range(BC):
            P2(b)
            if b + 2 < BC:
                P1(b + 2)

    nc.compile()
    return nc


# ----------------------------------------------------------------------------
# entry point
# ----------------------------------------------------------------------------

_CACHE = {}


def _get_prog():
    if "prog" not in _CACHE:
        import sys
        if "/opt/trn_rl_repo" not in sys.path:
            sys.path.insert(0, "/opt/trn_rl_repo")
        C = build_constants()
        nc = build_program(C)
        _CACHE["prog"] = (nc, C)
    return _CACHE["prog"]


def _make_in_maps(C, inputs):
    import ml_dtypes
    bf16 = ml_dtypes.bfloat16
    consts = {
        "CS": C["CS"].astype(bf16),
        "MI": C["MI"].astype(bf16),
        "WQ": C["WQ"].astype(bf16),
        "WT": C["WT"].astype(bf16),
        "PF": C["PF"].astype(np.float32),
        "UPM": np.pad(C["UPM"], ((0, 0), (0, 1))).astype(bf16),
        "WA": C["WA"].astype(bf16),
        "IDENT": np.eye(128, dtype=np.float32),
        "IDENTB": np.eye(128).astype(bf16),
        "EDGE_P": C["EDGE_P"].astype(bf16),
        "CHI": C["CHI"].reshape(2, 128, 2).astype(np.float32),
    }
    # TRI[k, p] = 1 iff k < p  (lhsT for exclusive prefix across partitions)
    consts["TRI"] = (np.arange(CW)[:, None] < np.arange(CW)[None, :]).astype(np.float32)

    in_maps = []
    for c in range(N_CORES):
        sl = slice(BC * c, BC * (c + 1))
        m = dict(consts)
        m["f0x3"] = _x3_of(inputs["f0"][sl, :, 0].astype(np.float32))
        m["oqx3"] = _x3_of(inputs["open_quotient"][sl, :, 0].astype(np.float32)).astype(bf16)
        m["amx3"] = _x3_of(inputs["amplitude"][sl, :, 0].astype(np.float32)).astype(bf16)
        m["vcur"] = inputs["vocal_tract_curve"][sl].astype(bf16)
        m["ncur"] = inputs["noise_filter_curve"][sl].astype(bf16)
        m["noise"] = inputs["noise"][sl].reshape(BC, NB, 256).astype(bf16)
        in_maps.append(m)
    return in_maps


def kernel(f0, amplitude, open_quotient, vocal_tract_curve, noise_filter_curve,
           noise):
    from concourse.bass_utils import run_bass_kernel_spmd
    nc, C = _get_prog()
    in_maps = _make_in_maps(C, dict(
        f0=f0, amplitude=amplitude, open_quotient=open_quotient,
        vocal_tract_curve=vocal_tract_curve,
        noise_filter_curve=noise_filter_curve, noise=noise))

    res = run_bass_kernel_spmd(nc, in_maps, list(range(N_CORES)))
    _CACHE["last_res"] = res
    out = np.concatenate(
        [res.results[c]["out"].reshape(BC, L) for c in range(N_CORES)], 0)
    return out.astype(np.float32)


if __name__ == "__main__":
    import sys
    sys.path.insert(0, "/opt/trn_rl_repo")
    C = build_constants()
    nc = build_program(C)
    print("program built and compiled OK")


# revision 19
# speedup vs baseline: 1.0654x; 1.0654x over previous
"""Trainium2 Bass kernel for nn_NeuralVoiceDecoder (self-contained).

kernel(**inputs) takes FULL inputs (batch 32), shards batch across 8
NeuronCores (4 rows each), runs one SPMD Bass program, gathers full output.

Algorithm / layouts (verified against the reference in fp64 at ~5e-4 rel):
  (b) "phase-major":  X2[c, R] = x[256*R + c], c in [0,256) as 2 halves of
      128 partitions, R in [0, NB).  STFT frames of hop 256 become pure
      column-shifted views of (b); overlap-add is PSUM column-shifted
      accumulation of the ISTFT matmuls.
  (c) "block-major":  chunks of 125 blocks: tile[p, jc] covers sample
      m = 256*(125*cc + p) + (jc-1), halo col jc=0 -> j=-1.
  q-packed rfft: q<513 -> RE bin q ; q in [513,1024) -> IM bin q-512
      (exactly 1024 rows: 513 RE + 511 nonzero IM).

Glottal source: phase cumsum distributes over the linear upsample, so the
within-block cumulative sums are a K=3 matmul with host-precomputed prefix
matrices; block offsets come from tiny triangular matmuls (all fp32 - the
phase is numerically chaotic downstream and must stay near-exact, matching
jax's tree-structured cumsum).  pulse*mask = sin^2(pi*w)*mask with
w = v - round(v) keeps the ACT Sin argument inside +-pi/2 where it is
accurate; floor(x) = int32_convert(x - 0.5) (convert rounds to nearest).
mask sigmoid is computed as 0.5 + 0.5*tanh(x/2): tanh and sin share one
ACT table set (silu_and_others) so no per-chunk table reloads.

Perf structure: rows are software-pipelined (stage P1 = excitation build,
vector/scalar-heavy; stage P2 = STFT*filter+ISTFT, tensor-dense) with P1
emitted two rows ahead so the PE stays continuously busy (HAM clock gate
needs sustained activity for the 2.4 GHz p-state).  Big matmuls run bf16
(1 cyc/row, same as f32r at N>=256, but half the SBUF -> triple-buffered
xp2) with fp32 PSUM accumulation; quantization adds ~4e-3 rel which is
well inside the 2e-2 gate.
"""
import os
import numpy as np

SR = 16000
N_FFT = 1024
HOP = 256
N_BINS = 65
B = 32
FRAMES = 1000
L = FRAMES * HOP
T = 1001
NB = 1000
NBP = 1004
CW = 125
NCH = 8
Q = 1024
HCOL = 257
N_CORES = 8
BC = B // N_CORES            # 4 rows per core
TCH = [(0, 512), (509, 492)]  # (t0, tn); tc i fills OA R_pad chunk [512i, 512i+512)
OUT_SPLIT = 510               # R_out boundary between OA chunk 0 and 1 (crop -2)


# ----------------------------------------------------------------------------
# host constants
# ----------------------------------------------------------------------------

def _hann(n=N_FFT):
    return 0.5 * (1.0 - np.cos(2.0 * np.pi * np.arange(n) / n))


def _triple(j):
    k = 10
    m = 256 * k + j
    pos = (m + 0.5) / 256.0 - 0.5
    i0 = int(np.floor(pos))
    w = pos - i0
    out = np.zeros(3)
    out[i0 - (k - 1)] += 1.0 - w
    out[i0 + 1 - (k - 1)] += w
    return out


def build_constants():
    C = {}
    w = _hann()

    UP = np.zeros((3, HCOL))
    for jc in range(HCOL):
        UP[:, jc] = _triple(jc - 1)
    C["UPM"] = UP

    PF = np.zeros((3, HCOL))
    acc = np.zeros(3)
    for jc in range(1, HCOL):
        acc = acc + _triple(jc - 1) / SR
        PF[:, jc] = acc
    C["PF"] = PF

    wsq_int = np.array([sum(w[256 * u + c] ** 2 for u in range(4)) for c in range(256)])
    wsq_int = np.maximum(wsq_int, 1e-11)
    WA = np.zeros((2, 3, 128))
    for h in range(2):
        for c in range(128):
            cf = 128 * h + c
            WA[h, :, c] = _triple(cf) / wsq_int[cf]
    C["WA"] = WA

    m_pad = np.arange(512, 512 + L)
    wsq_true = np.zeros(L)
    for u in range(-3, 4):
        t = m_pad // 256 + u
        s = m_pad - 256 * t
        valid = (t >= 0) & (t < T) & (s >= 0) & (s < 1024)
        wsq_true[valid] += w[s[valid]] ** 2
    wsq_true = np.maximum(wsq_true, 1e-11)
    ratio = wsq_int[np.arange(L) % 256] / wsq_true
    ratio_bR = ratio.reshape(NB, 256).T
    edge_cols = [R for R in range(NB)
                 if not np.allclose(ratio_bR[:, R], 1.0, atol=1e-13)]
    assert edge_cols == [0, NB - 1], edge_cols
    C["CHI"] = np.stack([ratio_bR[:, 0], ratio_bR[:, NB - 1]], 1)  # [256, 2]

    s = np.arange(N_FFT)
    CS = np.zeros((N_FFT, Q))
    for q in range(Q):
        if q < 513:
            CS[:, q] = w * np.cos(2 * np.pi * q * s / N_FFT)
        else:
            CS[:, q] = -w * np.sin(2 * np.pi * (q - 512) * s / N_FFT)
    C["CS"] = CS

    MI = np.zeros((Q, N_FFT))
    for q in range(Q):
        if q == 0:
            MI[q] = 1.0 / N_FFT
        elif q < 512:
            MI[q] = 2.0 / N_FFT * np.cos(2 * np.pi * q * s / N_FFT)
        elif q == 512:
            MI[q] = 1.0 / N_FFT * np.cos(np.pi * s)
        else:
            MI[q] = -2.0 / N_FFT * np.sin(2 * np.pi * (q - 512) * s / N_FFT)
    MI = MI * w[None, :]
    C["MI"] = MI

    Wt = np.zeros((FRAMES, T))
    for t in range(T):
        pos = (t + 0.5) * (FRAMES / T) - 0.5
        pos = min(max(pos, 0.0), FRAMES - 1.0)
        i0 = int(np.floor(pos)); i1 = min(i0 + 1, FRAMES - 1)
        wt = pos - i0
        Wt[i0, t] += 1.0 - wt
        Wt[i1, t] += wt
    C["WT"] = Wt

    Wq = np.zeros((N_BINS, Q))
    for q in range(Q):
        bq = q if q < 513 else q - 512
        pos = (bq + 0.5) * (N_BINS / 513.0) - 0.5
        pos = min(max(pos, 0.0), N_BINS - 1.0)
        i0 = int(np.floor(pos)); i1 = min(i0 + 1, N_BINS - 1)
        wq = pos - i0
        Wq[i0, q] += 1.0 - wq
        Wq[i1, q] += wq
    C["WQ"] = Wq

    # reflect-edge permutations, deduped.  For each (h, R_pad): list of
    # (src_half, src_R, mat_index); mats stacked in C["EDGE_P"].
    def edge_src(R_pad, cf):
        if R_pad < 2:
            return 512 - (256 * R_pad + cf)
        return L - 2 - (256 * (R_pad - 1002) + cf)
    mats = []
    mat_keys = {}
    edge_plan = {}
    for R_pad in [0, 1, 1002, 1003]:
        for h in range(2):
            blocks = {}
            for c in range(128):
                cf = 128 * h + c
                msrc = edge_src(R_pad, cf)
                assert 0 <= msrc < L
                Rs, cs = divmod(msrc, 256)
                hs, csl = divmod(cs, 128)
                if (hs, Rs) not in blocks:
                    blocks[(hs, Rs)] = np.zeros((128, 128), np.float32)
                blocks[(hs, Rs)][csl, c] = 1.0
            plan = []
            for (hs, Rs), P in blocks.items():
                key = P.tobytes()
                if key not in mat_keys:
                    mat_keys[key] = len(mats)
                    mats.append(P)
                plan.append((hs, Rs, mat_keys[key]))
            edge_plan[(h, R_pad)] = plan
    C["EDGE_P"] = np.stack(mats)          # [nP, 128, 128]
    C["edge_plan"] = edge_plan

    # Wt nonzero block list per t-chunk
    wt_blocks = {}
    for tci, (t0, tn) in enumerate(TCH):
        for kk in range(NCH):
            blk = Wt[CW * kk:CW * kk + CW, t0:t0 + tn]
            if np.any(blk):
                wt_blocks.setdefault(tci, []).append(kk)
    C["wt_blocks"] = wt_blocks
    return C


def _x3_of(x):          # [n, 1000] -> [n, 3, 1000] with edge clamping
    n = x.shape[0]
    x3 = np.zeros((n, 3, FRAMES), np.float32)
    x3[:, 0, 1:] = x[:, :-1]; x3[:, 0, 0] = x[:, 0]
    x3[:, 1] = x
    x3[:, 2, :-1] = x[:, 1:]; x3[:, 2, -1] = x[:, -1]
    return x3


# ----------------------------------------------------------------------------
# device program
# ----------------------------------------------------------------------------

def build_program(C):
    import concourse.bacc as bacc
    import concourse.tile as tile
    from concourse import mybir

    F32 = mybir.dt.float32
    F32R = mybir.dt.float32r
    BF16 = mybir.dt.bfloat16
    I32 = mybir.dt.int32
    ALU = mybir.AluOpType
    ACTF = mybir.ActivationFunctionType

    nc = bacc.Bacc("TRN2", target_bir_lowering=False, debug=False)

    # ---- dram I/O
    d_f0x3 = nc.dram_tensor("f0x3", [BC, 3, FRAMES], F32, kind="ExternalInput").ap()
    d_oqx3 = nc.dram_tensor("oqx3", [BC, 3, FRAMES], BF16, kind="ExternalInput").ap()
    d_amx3 = nc.dram_tensor("amx3", [BC, 3, FRAMES], BF16, kind="ExternalInput").ap()
    d_vcur = nc.dram_tensor("vcur", [BC, FRAMES, N_BINS], BF16, kind="ExternalInput").ap()
    d_ncur = nc.dram_tensor("ncur", [BC, FRAMES, N_BINS], BF16, kind="ExternalInput").ap()
    d_noise = nc.dram_tensor("noise", [BC, NB, 256], BF16, kind="ExternalInput").ap()
    d_CS = nc.dram_tensor("CS", [Q, Q], BF16, kind="ExternalInput").ap()
    d_MI = nc.dram_tensor("MI", [Q, Q], BF16, kind="ExternalInput").ap()
    d_WQ = nc.dram_tensor("WQ", [N_BINS, Q], BF16, kind="ExternalInput").ap()
    d_WT = nc.dram_tensor("WT", [FRAMES, T], BF16, kind="ExternalInput").ap()
    d_PF = nc.dram_tensor("PF", [3, HCOL], F32, kind="ExternalInput").ap()
    d_UPM = nc.dram_tensor("UPM", [3, HCOL + 1], BF16, kind="ExternalInput").ap()
    d_WA = nc.dram_tensor("WA", [2, 3, 128], BF16, kind="ExternalInput").ap()
    d_TRI = nc.dram_tensor("TRI", [CW, CW], F32, kind="ExternalInput").ap()
    d_ID = nc.dram_tensor("IDENT", [128, 128], F32, kind="ExternalInput").ap()
    d_IDB = nc.dram_tensor("IDENTB", [128, 128], BF16, kind="ExternalInput").ap()
    nP = C["EDGE_P"].shape[0]
    d_EP = nc.dram_tensor("EDGE_P", [nP, 128, 128], BF16, kind="ExternalInput").ap()
    d_CHI = nc.dram_tensor("CHI", [2, 128, 2], F32, kind="ExternalInput").ap()
    d_out = nc.dram_tensor("out", [BC, NB, 256], F32, kind="ExternalOutput").ap()

    from contextlib import ExitStack
    with tile.TileContext(nc) as tc, ExitStack() as es:
        cpool = es.enter_context(tc.tile_pool(name="consts", bufs=1))
        wpool = es.enter_context(tc.tile_pool(name="work", bufs=1))
        pp = es.enter_context(tc.tile_pool(name="psum", bufs=1, space="PSUM"))

        def ctile(shape, dt, name, src=None):
            t = cpool.tile(shape, dt, name=name, tag=name)
            if src is not None:
                nc.sync.dma_start(t[:], src)
            return t

        # ---- constants in SBUF
        WQt = ctile([N_BINS, Q], BF16, "WQt", d_WQ[:])
        PFt = ctile([3, HCOL], F32, "PFt", d_PF[:])
        UPt = ctile([3, HCOL + 1], BF16, "UPt", d_UPM[:])
        WAt = [ctile([3, 128], BF16, f"WA{h}", d_WA[h]) for h in range(2)]
        TRIt = ctile([CW, CW], F32, "TRIt", d_TRI[:])
        IDt = ctile([128, 128], F32, "IDt", d_ID[:])
        IDb = ctile([128, 128], BF16, "IDb", d_IDB[:])
        EPt = [ctile([128, 128], BF16, f"EP{i}", d_EP[i]) for i in range(nP)]
        CHIt = [ctile([128, 2], F32, f"CHI{h}", d_CHI[h]) for h in range(2)]

        ones125 = ctile([CW, 1], F32, "ones125"); nc.vector.memset(ones125[:], 1.0)
        ones1x = ctile([1, CW], F32, "ones1x"); nc.vector.memset(ones1x[:], 1.0)
        zl = ctile([1, 128], BF16, "zl"); nc.vector.memset(zl[:], 0.0)
        zr = ctile([1, 512], BF16, "zr"); nc.vector.memset(zr[:], 0.0)
        zcol = ctile([128, 2], BF16, "zcol"); nc.vector.memset(zcol[:], 0.0)
        zlf = ctile([1, CW], F32, "zlf"); nc.vector.memset(zlf[:], 0.0)
        zrf = ctile([1, 8], F32, "zrf"); nc.vector.memset(zrf[:], 0.0)
        cv = {}
        for val in (0.0, 50.0, np.pi):
            t = ctile([128, 1], F32, f"cv{len(cv)}")
            nc.vector.memset(t[:], float(val))
            cv[val] = t

        CSt = [ctile([128, Q], BF16, f"CS{k}", d_CS[128 * k:128 * k + 128, :])
               for k in range(8)]
        MIt = [ctile([128, Q], BF16, f"MI{k}", d_MI[128 * k:128 * k + 128, :])
               for k in range(8)]
        # resident WT blocks (bf16), one per (tci, kk) with nonzero data
        WTt = {}
        for tci, (t0, tn) in enumerate(TCH):
            for kk in C["wt_blocks"][tci]:
                WTt[(tci, kk)] = ctile(
                    [CW, tn], BF16, f"WT{tci}_{kk}",
                    d_WT[CW * kk:CW * kk + CW, t0:t0 + tn])

        # ---- per-row pipeline
        def wt_(shape, dt, name, tag, bufs=2):
            return wpool.tile(shape, dt, name=name, tag=tag, bufs=bufs)

        def pt_(shape, name, tag, bufs=None, dt=None):
            if bufs is None:
                bufs = {'px': 1, 'mm': 4, 'tp': 1, 'oa': 2}[tag]
            return pp.tile(shape, dt or mybir.dt.float32, name=name, tag=tag,
                           bufs=bufs)

        xp2 = {}    # (b, sig, h) -> tile [128, NBP] bf16
        ampw = {}   # (b, h) -> tile [128, NB] bf16
        curs = {}   # (b, tci, branch, kk) -> tile [CW, N_BINS] bf16

        def glottal_stage(b):
            # x3 tiles
            f0x3 = wt_([3, FRAMES], F32, f"f0x3_{b}", "x3f", 2)
            nc.sync.dma_start(f0x3[:], d_f0x3[b])
            oqx3 = wt_([3, FRAMES], BF16, f"oqx3_{b}", "x3o", 2)
            nc.sync.dma_start(oqx3[:], d_oqx3[b])

            # block totals G [CW, NCH] (fp32)
            g_ps = pt_([CW, 8], f"g_ps_{b}", "px")
            nc.tensor.matmul(g_ps[:], zlf[:, 0:CW], zrf[:], start=True, stop=False,
                             skip_group_check=True)
            for cc in range(NCH):
                nc.tensor.matmul(g_ps[:, cc:cc + 1], f0x3[:, CW * cc:CW * cc + CW],
                                 PFt[:, HCOL - 1:HCOL], start=False,
                                 stop=(cc == NCH - 1), skip_group_check=True)
            g_sb = wt_([CW, 8], F32, f"g_sb_{b}", "gsb", 2)
            nc.vector.tensor_copy(g_sb[:], g_ps[:])

            # P_excl = TRIstrict @ G + broadcast(base)
            h_ps = pt_([1, 8], f"h_ps_{b}", "px")
            nc.tensor.matmul(h_ps[:], ones125[:], g_sb[:], start=True, stop=True)
            h_sb = wt_([1, 8], F32, f"h_sb_{b}", "hsb", 2)
            nc.vector.tensor_copy(h_sb[:], h_ps[:])
            base = wt_([1, 8], F32, f"base_{b}", "base", 2)
            nc.vector.memset(base[:, 0:1], 0.0)
            nc.vector.tensor_tensor_scan(base[:, 1:8], h_sb[:, 0:7],
                                         zrf[0:1, 0:7], 0.0, ALU.add, ALU.bypass)
            p_ps = pt_([CW, 8], f"p_ps_{b}", "px")
            nc.tensor.matmul(p_ps[:], TRIt[:], g_sb[:], start=True, stop=False)
            nc.tensor.matmul(p_ps[:], ones1x[:], base[:], start=False, stop=True,
                             skip_group_check=True)
            p5 = wt_([CW, 8], F32, f"p5_{b}", "p5", 2)
            nc.vector.tensor_scalar(p5[:], p_ps[:], -0.5, None, ALU.add)

            # quarter-row elementwise, phase-batched so all 4 Tanh run
            # back-to-back and all 4 Sin run back-to-back (one ACT table
            # load each instead of one per switch).  Engine split:
            # ACT: ph5 (Identity+bias), oqe, th (Tanh), sn (Sin)
            # DVE: p, roq, u2, mask, varg, sm, g, sc-add
            # Pool: i32/i32v casts (round-to-nearest), marg, e
            W2 = 2 * HCOL
            ph5_, oqe_, p_, marg_, th_ = {}, {}, {}, {}, {}
            for qq in range(4):
                ccs = (2 * qq, 2 * qq + 1)
                ph5 = wt_([CW, W2], F32, f"ph5_{b}_{qq}", "ph5", 2)
                oqe = wt_([CW, W2], F32, f"oqe_{b}_{qq}", "oqe", 4)
                for i, cc in enumerate(ccs):
                    s_ps = pt_([CW, HCOL], f"s_ps_{b}_{cc}", "px")
                    nc.tensor.matmul(s_ps[:], f0x3[:, CW * cc:CW * cc + CW], PFt[:],
                                     start=True, stop=True)
                    nc.scalar.activation(ph5[:, HCOL * i:HCOL * (i + 1)], s_ps[:],
                                         ACTF.Identity, bias=p5[:, cc:cc + 1],
                                         scale=1.0)
                    o_ps = pt_([CW, HCOL + 1], f"o_ps_{b}_{cc}", "px")
                    nc.tensor.matmul(o_ps[:], oqx3[:, CW * cc:CW * cc + CW], UPt[:],
                                     start=True, stop=True)
                    nc.scalar.activation(oqe[:, HCOL * i:HCOL * (i + 1)],
                                         o_ps[:, 0:HCOL],
                                         ACTF.Copy, bias=1e-8, scale=1.0)
                i32 = wt_([CW, W2], I32, f"i32_{b}_{qq}", "i32", 2)
                nc.gpsimd.tensor_copy(i32[:], ph5[:])
                p = wt_([CW, W2], F32, f"p_{b}_{qq}", "pt", 4)
                nc.vector.scalar_tensor_tensor(p[:], ph5[:], 0.5, i32[:],
                                               ALU.add, ALU.subtract)
                marg = wt_([CW, W2], F32, f"marg_{b}_{qq}", "t1", 4)
                nc.gpsimd.tensor_tensor(marg[:], oqe[:], p[:], ALU.subtract)
                ph5_[qq], oqe_[qq], p_[qq], marg_[qq] = ph5, oqe, p, marg
            for qq in range(4):
                # mask = sigmoid(100*marg) = 0.5 + 0.5*tanh(50*marg); the 4
                # Tanh run back-to-back (one table load), and the only ACT
                # ops until the Sin below are Copy/Identity (table-free)
                th = wt_([CW, W2], F32, f"th_{b}_{qq}", "t2", 4)
                nc.scalar.activation(th[:], marg_[qq][:], ACTF.Tanh,
                                     bias=cv[0.0][0:CW, :], scale=cv[50.0][0:CW, :])
                th_[qq] = th
            for qq in range(4):
                ccs = (2 * qq, 2 * qq + 1)
                mask = wt_([CW, W2], F32, f"mask_{b}_{qq}", "t4", 2)
                nc.vector.tensor_scalar(mask[:], th_[qq][:], 0.5, 0.5,
                                        ALU.mult, ALU.add)
                roq = wt_([CW, W2], F32, f"roq_{b}_{qq}", "t58", 2)
                nc.vector.reciprocal_approx_fast(roq[:], oqe_[qq][:])
                u2 = wt_([CW, W2], F32, f"u2_{b}_{qq}", "t3", 2)
                nc.vector.scalar_tensor_tensor(u2[:], p_[qq][:], 0.5, roq[:],
                                               ALU.mult, ALU.mult)
                i32v = wt_([CW, W2], I32, f"i32v_{b}_{qq}", "i32", 2)
                nc.gpsimd.tensor_copy(i32v[:], u2[:])
                varg = wt_([CW, W2], F32, f"varg_{b}_{qq}", "t6", 2)
                nc.vector.tensor_tensor(varg[:], u2[:], i32v[:], ALU.subtract)
                sn = wt_([CW, W2], F32, f"sn_{b}_{qq}", "t7", 2)
                nc.scalar.activation(sn[:], varg[:], ACTF.Sin,
                                     bias=cv[0.0][0:CW, :], scale=cv[np.pi][0:CW, :])
                sm = wt_([CW, W2], F32, f"sm_{b}_{qq}", "t1", 4)
                nc.vector.tensor_tensor(sm[:], sn[:], mask[:], ALU.mult)
                g = wt_([CW, W2], F32, f"g_{b}_{qq}", "t58", 2)
                nc.vector.tensor_tensor(g[:], sm[:], sn[:], ALU.mult)
                e = wt_([CW, 512], BF16, f"e_{b}_{qq}", "et", 2)
                for i, cc in enumerate(ccs):
                    nc.gpsimd.tensor_tensor(e[:, 256 * i:256 * (i + 1)],
                                            g[:, HCOL * i + 1:HCOL * i + HCOL],
                                            g[:, HCOL * i:HCOL * i + 256],
                                            ALU.subtract)
                if qq == 0:
                    nc.gpsimd.memset(e[0:1, 0:1], 0.0)
                # transpose (c)->(b): e[:, 256i + 128h2 : +128] -> xp2 cols
                for i, cc in enumerate(ccs):
                    for h2 in range(2):
                        tp = pt_([128, CW], f"tp_{b}_{cc}_{h2}", "tp",
                                 dt=BF16)
                        nc.tensor.transpose(tp[:], e[:, 256 * i + 128 * h2:
                                                     256 * i + 128 * (h2 + 1)],
                                            IDb[0:CW, 0:CW])
                        dst = xp2[(b, "g", h2)]
                        if (cc + h2) % 2 == 0:
                            nc.vector.tensor_copy(dst[:, 2 + CW * cc:2 + CW * cc + CW], tp[:])
                        else:
                            nc.scalar.copy(dst[:, 2 + CW * cc:2 + CW * cc + CW], tp[:])

        def noise_stage(b):
            for cc in range(NCH):
                nb_t = wt_([CW, 256], BF16, f"nb_{b}_{cc}", "nb", 2)
                nc.sync.dma_start(nb_t[:], d_noise[b, CW * cc:CW * cc + CW, :])
                for h2 in range(2):
                    tp = pt_([128, CW], f"ntp_{b}_{cc}_{h2}", "tp", dt=BF16)
                    nc.tensor.transpose(tp[:], nb_t[:, 128 * h2:128 * (h2 + 1)],
                                        IDb[0:CW, 0:CW])
                    dst = xp2[(b, "n", h2)]
                    if (cc + h2) % 2 == 0:
                        nc.vector.tensor_copy(dst[:, 2 + CW * cc:2 + CW * cc + CW], tp[:])
                    else:
                        nc.scalar.copy(dst[:, 2 + CW * cc:2 + CW * cc + CW], tp[:])

        def edge_stage(b, sig):
            for (h, R_pad), plan in C["edge_plan"].items():
                off = 0 if R_pad < 2 else 1
                ep = pt_([128, 2], f"ep_{b}_{sig}_{h}_{R_pad}", "px")
                for i, (hs, Rs, mi) in enumerate(plan):
                    sv = xp2[(b, sig, hs)][:, 2 + Rs - off:4 + Rs - off]
                    nc.tensor.matmul(ep[:], EPt[mi][:], sv, start=(i == 0),
                                     stop=(i == len(plan) - 1),
                                     skip_group_check=True)
                dst = xp2[(b, sig, h)]
                nc.vector.tensor_copy(dst[:, R_pad:R_pad + 1], ep[:, off:off + 1])

        def ampw_stage(b):
            amx3 = wt_([3, FRAMES], BF16, f"amx3_{b}", "x3a", 2)
            nc.sync.dma_start(amx3[:], d_amx3[b])
            for h in range(2):
                aw = wt_([128, NB], BF16, f"ampw_{b}_{h}", f"ampw{h}", 3)
                for half in range(2):
                    a_ps = pt_([128, 500], f"a_ps_{b}_{h}_{half}", "px")
                    nc.tensor.matmul(a_ps[:], WAt[h][:],
                                     amx3[:, 500 * half:500 * (half + 1)],
                                     start=True, stop=True)
                    # fold chi edge correction into cols 0 and 999
                    if half == 0:
                        nc.vector.tensor_scalar(aw[:, 0:1], a_ps[:, 0:1],
                                                CHIt[h][:, 0:1], None, ALU.mult)
                        nc.vector.tensor_copy(aw[:, 1:500], a_ps[:, 1:500])
                    else:
                        nc.vector.tensor_copy(aw[:, 500:999], a_ps[:, 0:499])
                        nc.vector.tensor_scalar(aw[:, 999:1000], a_ps[:, 499:500],
                                                CHIt[h][:, 1:2], None, ALU.mult)
                ampw[(b, h)] = aw

        def cur_prefetch(b):
            for tci in range(2):
                for branch, d_cur in (("v", d_vcur), ("n", d_ncur)):
                    for kk in C["wt_blocks"][tci]:
                        cur = wt_([CW, N_BINS], BF16,
                                  f"cur_{b}_{tci}_{branch}_{kk}",
                                  f"cur{tci}{branch}{kk}", 3)
                        nc.sync.dma_start(cur[:], d_cur[b, CW * kk:CW * kk + CW, :])
                        curs[(b, tci, branch, kk)] = cur

        def P1(b):
            for sig in ("g", "n"):
                for h in range(2):
                    xp2[(b, sig, h)] = wt_([128, NBP], BF16, f"xp2{sig}{h}_{b}",
                                           f"xp2{sig}{h}", 3)
            cur_prefetch(b)
            noise_stage(b)
            glottal_stage(b)
            edge_stage(b, "g")
            edge_stage(b, "n")
            ampw_stage(b)

        def gt_stage(b, tci, branch):
            t0, tn = TCH[tci]
            gt_ps = pt_([N_BINS, tn], f"gt_ps_{b}_{tci}_{branch}", "mm")
            kks = C["wt_blocks"][tci]
            for i, kk in enumerate(kks):
                nc.tensor.matmul(gt_ps[:], curs[(b, tci, branch, kk)][:],
                                 WTt[(tci, kk)][:],
                                 start=(i == 0), stop=(i == len(kks) - 1))
            gt_sb = wt_([N_BINS, tn], BF16, f"gt_sb_{b}_{tci}_{branch}",
                        f"gtsb{branch}", 2)
            nc.scalar.copy(gt_sb[:], gt_ps[:])
            return gt_sb

        def filtspec_stage(b, tci):
            t0, tn = TCH[tci]
            gt = {"g": gt_stage(b, tci, "v"), "n": gt_stage(b, tci, "n")}
            spec_c = []
            for mm in range(8):
                f_ps = {}
                for sig in ("g", "n"):
                    fp = pt_([128, tn], f"f_ps_{b}_{tci}_{sig}_{mm}", "mm")
                    nc.tensor.matmul(fp[:], WQt[:, 128 * mm:128 * (mm + 1)],
                                     gt[sig][:], start=True, stop=True)
                    f_ps[sig] = fp
                # evict filt to SBUF (only one PSUM operand allowed per
                # elementwise op; the spectrum stays in PSUM)
                f_sb = {}
                for sig in ("g", "n"):
                    fs = wt_([128, tn], BF16, f"f_sb_{b}_{tci}_{sig}_{mm}",
                             f"fsb{sig}", 2)
                    nc.scalar.copy(fs[:], f_ps[sig][:])
                    f_sb[sig] = fs
                sp_ps = {s: pt_([128, tn], f"sp_ps_{b}_{tci}_{s}_{mm}", "mm")
                         for s in ("g", "n")}
                for kk in range(8):
                    for sig in ("g", "n"):
                        rhs = xp2[(b, sig, kk % 2)][:, kk // 2 + t0:kk // 2 + t0 + tn]
                        nc.tensor.matmul(sp_ps[sig][:],
                                         CSt[kk][:, 128 * mm:128 * (mm + 1)],
                                         rhs, start=(kk == 0), stop=(kk == 7))
                # tm = spec*filt on DVE (one PSUM operand each); the g+n
                # add runs on GpSimd (SBUF-only engine)
                tma = wt_([128, tn], BF16, f"tma_{b}_{tci}_{mm}", "tma", 2)
                nc.vector.tensor_tensor(tma[:], sp_ps["g"][:], f_sb["g"][:], ALU.mult)
                tmb = wt_([128, tn], BF16, f"tmb_{b}_{tci}_{mm}", "tmb", 2)
                nc.vector.tensor_tensor(tmb[:], sp_ps["n"][:], f_sb["n"][:], ALU.mult)
                sc = wt_([128, tn + 2], BF16, f"sc_{b}_{tci}_{mm}", f"sc{mm}", 1)
                nc.vector.tensor_copy(sc[:, 0:1], zcol[:, 0:1])
                nc.vector.tensor_copy(sc[:, tn + 1:tn + 2], zcol[:, 1:2])
                nc.vector.tensor_tensor(sc[:, 1:tn + 1], tma[:], tmb[:], ALU.add)
                spec_c.append(sc)
            return spec_c

        def istft_stage(b, tci, spec_c, oa):
            t0, tn = TCH[tci]
            rc = tci
            Clo = 512 * rc
            Chi = min(Clo + 512, NBP)
            for h in range(2):
                nc.tensor.matmul(oa[h][:, 0:Chi - Clo], zl[:], zr[:, 0:Chi - Clo],
                                 start=True, stop=False, skip_group_check=True)
            spans = {}
            for mm in range(8):
                u, h = divmod(mm, 2)
                lo = max(t0 + u, Clo)
                hi = min(t0 + tn + u, Chi)
                if lo >= hi:
                    continue
                if (hi - lo) % 2 == 1:
                    if rc == 0:
                        lo -= 1      # extra col reads the leading zero column
                    else:
                        hi += 1      # extra col reads the trailing zero column
                a = 1 + lo - u - t0
                bcol = 1 + hi - u - t0
                assert 0 <= a and bcol <= tn + 2 and Clo <= lo and hi <= 512 * rc + 512
                spans[mm] = (h, lo, hi, a, bcol)
            last_mm = {h: max(m for m in spans if spans[m][0] == h) for h in range(2)}
            # kk-outer: start accumulating as soon as spec_c[kk] is ready
            for kk in range(8):
                for mm, (h, lo, hi, a, bcol) in spans.items():
                    nc.tensor.matmul(oa[h][:, lo - Clo:hi - Clo],
                                     MIt[kk][:, 128 * mm:128 * (mm + 1)],
                                     spec_c[kk][:, a:bcol],
                                     start=False,
                                     stop=(kk == 7 and mm == last_mm[h]),
                                     skip_group_check=True)

        def final_stage(b, tci, oa):
            rc = tci
            # out_b = OA * ampw  (slices per rc)
            for h in range(2):
                if rc == 0:
                    w_ = OUT_SPLIT
                    ob = wt_([128, w_], BF16, f"ob_{b}_{rc}_{h}", f"ob{h}", 2)
                    nc.vector.tensor_tensor(ob[:], oa[h][:, 2:2 + w_],
                                            ampw[(b, h)][:, 0:w_], ALU.mult)
                else:
                    w_ = NB - OUT_SPLIT
                    ob = wt_([128, w_], BF16, f"ob_{b}_{rc}_{h}", f"ob{h}", 2)
                    nc.vector.tensor_tensor(ob[:], oa[h][:, 0:w_],
                                            ampw[(b, h)][:, OUT_SPLIT:NB], ALU.mult)
                # transpose to (c) pieces and DMA out
                if rc == 0:
                    bnds = [0, 128, 256, 384, OUT_SPLIT]
                    base = 0
                else:
                    bnds = [0, 128, 256, 384, NB - OUT_SPLIT]
                    base = OUT_SPLIT
                for i in range(len(bnds) - 1):
                    a, bb = bnds[i], bnds[i + 1]
                    wb = bb - a
                    tp = pt_([wb, 128], f"otp_{b}_{rc}_{h}_{i}", "tp", dt=BF16)
                    nc.tensor.transpose(tp[:], ob[:, a:bb], IDb[:])
                    st = wt_([wb, 128], F32, f"ost_{b}_{rc}_{h}_{i}", "ost", 2)
                    if i % 2 == 0:
                        nc.vector.tensor_copy(st[:], tp[:])
                    else:
                        nc.scalar.copy(st[:], tp[:])
                    nc.sync.dma_start(
                        d_out[b, base + a:base + bb, 128 * h:128 * (h + 1)], st[:])

        def P2(b):
            # final(tci0) is deferred past filtspec(tci1) so the PE rolls
            # straight from ISTFT(0) into the next chunk's matmuls while the
            # DVE does the amp-multiply eviction of OA(0)
            spec0 = filtspec_stage(b, 0)
            oa0 = {h: pt_([128, 512], f"oa_{b}_0_{h}", "oa", 2)
                   for h in range(2)}
            istft_stage(b, 0, spec0, oa0)
            spec1 = filtspec_stage(b, 1)
            final_stage(b, 0, oa0)
            oa1 = {h: pt_([128, 512], f"oa_{b}_1_{h}", "oa", 2)
                   for h in range(2)}
            istft_stage(b, 1, spec1, oa1)
            final_stage(b, 1, oa1)

        # ---- main loop: software-pipelined rows (P1 two rows ahead of
        # the tensor-dense P2)
        P1(0)
        if BC > 1:
            P1(1)
        for b in range(BC):
            P2(b)
            if b + 2 < BC:
                P1(b + 2)

    nc.compile()
    return nc


# ----------------------------------------------------------------------------
# entry point
# ----------------------------------------------------------------------------

_CACHE = {}


def _get_prog():
    if "prog" not in _CACHE:
        import sys
        if "/opt/trn_rl_repo" not in sys.path:
            sys.path.insert(0, "/opt/trn_rl_repo")
        C = build_constants()
        nc = build_program(C)
        _CACHE["prog"] = (nc, C)
    return _CACHE["prog"]


def _make_in_maps(C, inputs):
    import ml_dtypes
    bf16 = ml_dtypes.bfloat16
    consts = {
        "CS": C["CS"].astype(bf16),
        "MI": C["MI"].astype(bf16),
        "WQ": C["WQ"].astype(bf16),
        "WT": C["WT"].astype(bf16),
        "PF": C["PF"].astype(np.float32),
        "UPM": np.pad(C["UPM"], ((0, 0), (0, 1))).astype(bf16),
        "WA": C["WA"].astype(bf16),
        "IDENT": np.eye(128, dtype=np.float32),
        "IDENTB": np.eye(128).astype(bf16),
        "EDGE_P": C["EDGE_P"].astype(bf16),
        "CHI": C["CHI"].reshape(2, 128, 2).astype(np.float32),
    }
    # TRI[k, p] = 1 iff k < p  (lhsT for exclusive prefix across partitions)
    consts["TRI"] = (np.arange(CW)[:, None] < np.arange(CW)[None, :]).astype(np.float32)

    in_maps = []
    for c in range(N_CORES):
        sl = slice(BC * c, BC * (c + 1))
        m = dict(consts)
        m["f0x3"] = _x3_of(inputs["f0"][sl, :, 0].astype(np.float32))
        m["oqx3"] = _x3_of(inputs["open_quotient"][sl, :, 0].astype(np.float32)).astype(bf16)
        m["amx3"] = _x3_of(inputs["amplitude"][sl, :, 0].astype(np.float32)).astype(bf16)
        m["vcur"] = inputs["vocal_tract_curve"][sl].astype(bf16)
        m["ncur"] = inputs["noise_filter_curve"][sl].astype(bf16)
        m["noise"] = inputs["noise"][sl].reshape(BC, NB, 256).astype(bf16)
        in_maps.append(m)
    return in_maps


def kernel(f0, amplitude, open_quotient, vocal_tract_curve, noise_filter_curve,
           noise):
    from concourse.bass_utils import run_bass_kernel_spmd
    nc, C = _get_prog()
    in_maps = _make_in_maps(C, dict(
        f0=f0, amplitude=amplitude, open_quotient=open_quotient,
        vocal_tract_curve=vocal_tract_curve,
        noise_filter_curve=noise_filter_curve, noise=noise))

    res = run_bass_kernel_spmd(nc, in_maps, list(range(N_CORES)))
    _CACHE["last_res"] = res
    out = np.concatenate(
        [res.results[c]["out"].reshape(BC, L) for c in range(N_CORES)], 0)
    return out.astype(np.float32)


if __name__ == "__main__":
    import sys
    sys.path.insert(0, "/opt/trn_rl_repo")
    C = build_constants()
    nc = build_program(C)
    print("program built and compiled OK")
